# revision 24
# baseline (speedup 1.0000x reference)
"""Trainium2 Bass kernel for CAGNN (GAT-style) message passing, 8 NeuronCores.

Strategy (edge-parallel, dst-sharded, zero collectives, identity-PE):
  - Each core owns 12,500 destination nodes (1/8 slice). Host sorts each
    core's nodes by in-degree and lays each node's incoming edges in a
    [128-node chunk x slot] grid (common slot profile across cores so all
    8 cores run one SPMD program).
  - Device program 1 (8-way sharded): T = [feat @ W | el | er] where
    el = ft . attn_l, er = ft . attn_r (el = feat @ (W @ attn_l)).
  - Host gathers per-slot [ft | 1 | 1] rows (bf16) plus pair-duplicated
    el/er streams — index copies only, no arithmetic.
  - Device program 2: x2 = exp(leaky_relu(el2 + er2)) computed in a few
    wide ops. Per chunk ONE wide DVE tensor_tensor scales all K slots:
    fw = rows * x_broadcast (the pair-duplicated x layout makes the
    broadcast AP's innermost dim [1,2] so the DVE runs in its fast mode),
    then TensorE accumulates PSUM += I.T @ fw_k per slot (identity
    stationary weights: PE is a pure PSUM accumulator, ~40ns/slot).
    Epilogue in 4-chunk windows: batched max/reciprocal on the PSUM
    denominator column, per-chunk ACT scaled PSUM->SBUF copy, one batched
    residual add, one batched output DMA.
  - Softmax max-subtraction is skipped: e is O(10), exp() is safe in f32,
    and a = exp(e)/sum(exp(e)) is mathematically identical.
"""
import sys

sys.path.insert(0, "/opt/trn_rl_repo")

import numpy as np
import ml_dtypes
from bass_rust import AP
import concourse.bass as bass
import concourse.tile as tile
from concourse import bacc, mybir
from concourse.bass2jax import run_bass_via_pjrt

P = 128
N_NODES = 100000
N_EDGES = 1600000
D = 64
WCOL = D + 2                                  # [ft(64) | 1 | 1]
N_CORES = 8
NODES_PER_CORE = N_NODES // N_CORES           # 12500
CHUNKS = (NODES_PER_CORE + P - 1) // P        # 98
GRID = CHUNKS * P                             # 12544 (44 pad)
T1_TILES = CHUNKS
T1_GRID = T1_TILES * P
NEG_SLOPE = 0.2
WIN = 4                                       # epilogue window (chunks)
BF16 = ml_dtypes.bfloat16

_cache = {}


def _build_program1():
    """T-build: tout2[66, n] = [ft | el | er].T for the core's node slice.

    One matmul per 128-node tile with a FIXED stationary [W | wl | wr]
    (loaded once); 512-col PSUM banks drained by ACT in 4-tile batches.
    """
    nc = bacc.Bacc("TRN2", target_bir_lowering=False, debug=False,
                   num_devices=N_CORES)
    featT = nc.dram_tensor("featT", [D, T1_GRID], mybir.dt.float32,
                           kind="ExternalInput")
    wall = nc.dram_tensor("wall", [D, D + 2], mybir.dt.float32,
                          kind="ExternalInput")
    tout2 = nc.dram_tensor("tout2", [D + 2, T1_GRID], mybir.dt.float32,
                           kind="ExternalOutput")
    with tile.TileContext(nc) as tc:
        with (tc.tile_pool(name="sb", bufs=3) as sb,
              tc.tile_pool(name="ob", bufs=3) as ob,
              tc.tile_pool(name="ps", bufs=4, space="PSUM") as ps,
              tc.tile_pool(name="pers", bufs=1) as pers):
            w_t = pers.tile([D, D + 2], mybir.dt.float32)
            nc.sync.dma_start(w_t[:], wall[:, :])
            B = 4                               # tiles per PSUM bank
            for t0 in range(0, T1_TILES, B):
                nb = min(B, T1_TILES - t0)
                ftT = sb.tile([D, B * P], mybir.dt.float32, tag="ftT")
                nc.sync.dma_start(ftT[:, 0:nb * P],
                                  featT[:, t0 * P:(t0 + nb) * P])
                psum = ps.tile([D + 2, 512], mybir.dt.float32, space="PSUM",
                               tag="t")
                nc.tensor.matmul(psum[:, 0:nb * P], lhsT=w_t[:],
                                 rhs=ftT[:, 0:nb * P],
                                 start=True, stop=True)
                row = ob.tile([D + 2, B * P], mybir.dt.float32, tag="row")
                nc.scalar.copy(row[:, 0:nb * P], psum[:, 0:nb * P])
                nc.sync.dma_start(tout2[:, t0 * P:(t0 + nb) * P],
                                  row[:, 0:nb * P])
    nc.finalize()
    return nc


def _build_program2(slot_counts):
    """Identity-PE aggregation pass. slot_counts[ch] = slots in chunk ch."""
    slot_counts = [int(s) for s in slot_counts]
    stot = sum(slot_counts)
    s2 = 2 * stot
    nc = bacc.Bacc("TRN2", target_bir_lowering=False, debug=False,
                   num_devices=N_CORES)
    rows = nc.dram_tensor("rows", [P, stot * WCOL], mybir.dt.bfloat16,
                          kind="ExternalInput")
    el2 = nc.dram_tensor("el2", [P, s2], mybir.dt.float16,
                         kind="ExternalInput")
    er2 = nc.dram_tensor("er2", [P, s2], mybir.dt.float16,
                         kind="ExternalInput")
    idn = nc.dram_tensor("idn", [P, P], mybir.dt.bfloat16,
                         kind="ExternalInput")
    fres = nc.dram_tensor("fres", [CHUNKS, P, D], mybir.dt.bfloat16,
                          kind="ExternalInput")
    out = nc.dram_tensor("out", [CHUNKS, P, D], mybir.dt.bfloat16,
                         kind="ExternalOutput")
    # chunk -> slot offset; x-tile boundaries aligned to chunk starts
    s_off = [0]
    for kk in slot_counts:
        s_off.append(s_off[-1] + kk)
    XT = 512                       # x-tile max width (slots)
    RG = WIN                       # rows-DMA group (chunks)
    rbounds = [0, 1, 2, 4, 8]
    while rbounds[-1] < CHUNKS:
        rbounds.append(min(rbounds[-1] + RG, CHUNKS))
    # x-tiles = unions of rows-groups (staircase targets, then XT)
    xtiles = []
    targets = [64, 192, 448]
    ti = 0
    i = 0
    while i < len(rbounds) - 1:
        tgt = targets[ti] if ti < len(targets) else XT
        ti += 1
        j = i + 1
        while (j < len(rbounds) - 1
               and s_off[rbounds[j]] - s_off[rbounds[i]] < tgt):
            j += 1
        xtiles.append((rbounds[i], rbounds[j]))
        i = j
    xt_max = max(s_off[b] - s_off[a] for a, b in xtiles)

    with tile.TileContext(nc) as tc:
        with (tc.tile_pool(name="rp", bufs=2) as rp,
              tc.tile_pool(name="fp", bufs=2) as fp,
              tc.tile_pool(name="xp", bufs=4) as xp,
              tc.tile_pool(name="ep", bufs=3) as ep,
              tc.tile_pool(name="wp", bufs=3) as wp,
              tc.tile_pool(name="ps", bufs=8, space="PSUM") as ps,
              tc.tile_pool(name="sp", bufs=3) as sp,
              tc.tile_pool(name="pers", bufs=1) as pers):
            i_t = pers.tile([P, P], mybir.dt.bfloat16)
            nc.sync.dma_start(i_t[:], idn[:, :])
            el_t = pers.tile([P, s2], mybir.dt.float16)
            er_t = pers.tile([P, s2], mybir.dt.float16)
            hd = min(256, s2)
            nc.sync.dma_start(el_t[:, 0:hd], el2[:, 0:hd])
            nc.sync.dma_start(er_t[:, 0:hd], er2[:, 0:hd])
            h = s2 // 2
            nc.sync.dma_start(el_t[:, hd:h], el2[:, hd:h])
            nc.scalar.dma_start(el_t[:, h:s2], el2[:, h:s2])
            nc.sync.dma_start(er_t[:, hd:h], er2[:, hd:h])
            nc.scalar.dma_start(er_t[:, h:s2], er2[:, h:s2])
            fres_t = pers.tile([P, CHUNKS * D], mybir.dt.bfloat16)
            fsrc_all = AP(fres[0].tensor, 0, [[D, P], [P * D, CHUNKS], [1, D]])
            nc.scalar.dma_start(fres_t[:], fsrc_all)

            cur_e = [None, -1]       # (tile, xtile idx)

            def emit_xtile(ti):
                """e = leaky_relu(el2 + er2) for the tile's slot range."""
                clo, chi = xtiles[ti]
                t0, t1 = 2 * s_off[clo], 2 * s_off[chi]
                tw = t1 - t0
                e_s = ep.tile([P, 2 * xt_max], mybir.dt.float16, tag="e")
                nc.vector.tensor_add(e_s[:, 0:tw], el_t[:, t0:t1],
                                     er_t[:, t0:t1])
                nc.vector.scalar_tensor_tensor(
                    out=e_s[:, 0:tw], in0=e_s[:, 0:tw], scalar=NEG_SLOPE,
                    in1=e_s[:, 0:tw],
                    op0=mybir.AluOpType.mult, op1=mybir.AluOpType.max)
                x2_s = xp.tile([P, 2 * xt_max], mybir.dt.bfloat16, tag="x2")
                nc.scalar.activation(x2_s[:, 0:tw], e_s[:, 0:tw],
                                     mybir.ActivationFunctionType.Exp)
                return x2_s

            def epilogue(w0, w_psums, rec_w):
                """Batched scale+residual for chunks w0..w0+nw-1."""
                nw = len(w_psums)
                t_w = sp.tile([P, WIN * D], mybir.dt.bfloat16, tag="t")
                for i, psum in enumerate(w_psums):
                    nc.scalar.activation(t_w[:, i * D:(i + 1) * D],
                                         psum[:, 0:D],
                                         mybir.ActivationFunctionType.Copy,
                                         scale=rec_w[:, i:i + 1])
                o_w = sp.tile([P, WIN * D], mybir.dt.bfloat16, tag="o")
                nc.vector.tensor_add(o_w[:, 0:nw * D], t_w[:, 0:nw * D],
                                     fres_t[:, w0 * D:(w0 + nw) * D])
                odst = AP(out[0].tensor, w0 * P * D,
                          [[D, P], [P * D, nw], [1, D]])
                nc.gpsimd.dma_start(odst, o_w[:, 0:nw * D])

            # lag-1 window pipeline: emit window w's epilogue after window
            # w+1's multiplies so no engine waits on the PE in-line
            ready = None
            pend_w0 = None
            pend_psums = []
            den_w = None
            pend_e = None
            next_xt = 0
            rg_max = max(s_off[b] - s_off[a]
                         for a, b in zip(rbounds, rbounds[1:]))
            rstarts = set(rbounds[:-1])
            rg_bound = dict(zip(rbounds, rbounds[1:]))
            rg_tile = None
            fw_g = None
            rg_lo = -1
            for ch in range(CHUNKS):
                kk = slot_counts[ch]
                s0 = s_off[ch]
                while next_xt < len(xtiles) and xtiles[next_xt][0] <= ch + 6:
                    nxt = [emit_xtile(next_xt), next_xt]
                    if next_xt == 0:
                        cur_e = nxt
                    else:
                        pend_e = nxt
                    next_xt += 1
                if cur_e[1] >= 0 and ch >= xtiles[cur_e[1]][1]:
                    cur_e = pend_e
                if ch in rstarts:
                    glo, ghi = ch, rg_bound[ch]
                    gk = s_off[ghi] - s_off[glo]
                    gw = gk * WCOL
                    rg_tile = rp.tile([P, rg_max * WCOL],
                                      mybir.dt.bfloat16, tag="rows")
                    gh = (gw // 2) & ~1
                    r0 = s_off[glo] * WCOL
                    nc.sync.dma_start(rg_tile[:, 0:gh],
                                      rows[:, r0:r0 + gh])
                    nc.scalar.dma_start(rg_tile[:, gh:gw],
                                        rows[:, r0 + gh:r0 + gw])
                    rg_lo = glo
                    # one wide fw = rows * x_broadcast for the whole group
                    x2g = cur_e[0]
                    xlo = xtiles[cur_e[1]][0]
                    goff = 2 * (s_off[glo] - s_off[xlo])
                    fw_g = fp.tile([P, rg_max * WCOL], mybir.dt.bfloat16,
                                   tag="fw")
                    xbg = AP(x2g[:].tensor, goff,
                             [[2 * xt_max, P], [2, gk], [0, WCOL // 2],
                              [1, 2]])
                    nc.vector.tensor_mul(fw_g[:, 0:gw],
                                         rg_tile[:, 0:gw], xbg)
                if pend_w0 is None:
                    pend_w0 = ch
                    den_w = wp.tile([P, WIN], mybir.dt.float32, tag="den")
                foff = (s0 - s_off[rg_lo]) * WCOL
                psum = ps.tile([P, 512], mybir.dt.float32, space="PSUM",
                               tag="acc")
                for k in range(kk):
                    nc.tensor.matmul(
                        psum[:, 0:WCOL], lhsT=i_t[:],
                        rhs=fw_g[:, foff + k * WCOL:foff + (k + 1) * WCOL],
                        start=(k == 0), stop=(k == kk - 1))
                # denominator (2*sum_k x) sits in PSUM col D (ones column)
                nc.scalar.copy(den_w[:, ch - pend_w0:ch - pend_w0 + 1],
                               psum[:, D:D + 1])
                pend_psums.append(psum)
                if len(pend_psums) == WIN or ch == CHUNKS - 1:
                    # rec = 1 / max(den/2, eps): depends only on the exps
                    nw = len(pend_psums)
                    nc.vector.tensor_scalar_max(den_w[:, 0:nw],
                                                den_w[:, 0:nw], 1e-30)
                    rec_w = wp.tile([P, WIN], mybir.dt.float32, tag="rec")
                    nc.vector.reciprocal(rec_w[:, 0:nw], den_w[:, 0:nw])
                    if ready is not None:
                        epilogue(*ready)
                    ready = (pend_w0, pend_psums, rec_w)
                    pend_w0, pend_psums = None, []
            if ready is not None:
                epilogue(*ready)
    nc.finalize()
    return nc


def _preprocess(src, dst):
    """Edge layout: per-core degree-sorted chunk/slot grid, common profile.

    Returns (perm[core][GRID] node-ids with -1 pads, slot_counts[CHUNKS],
    slot_src[core] int64 [total_slots, P] with -1 for pad slots).
    """
    deg = np.bincount(dst, minlength=N_NODES)
    order = np.argsort(dst, kind="stable")
    src_by_dst = src[order]
    rptr = np.zeros(N_NODES + 1, np.int64)
    np.cumsum(deg, out=rptr[1:])

    perms = []
    percore_counts = np.zeros((N_CORES, CHUNKS), np.int64)
    for c in range(N_CORES):
        lo = c * NODES_PER_CORE
        nodes = np.arange(lo, lo + NODES_PER_CORE)
        p = nodes[np.argsort(deg[nodes], kind="stable")]
        grid = np.full(GRID, -1, np.int64)
        grid[GRID - NODES_PER_CORE:] = p          # pads first (low-deg end)
        perms.append(grid)
        g = grid.reshape(CHUNKS, P)
        for ch in range(CHUNKS):
            real = g[ch][g[ch] >= 0]
            percore_counts[c, ch] = deg[real].max() if len(real) else 0
    slot_counts = np.maximum(1, percore_counts.max(axis=0))

    slot_srcs = []
    total = int(slot_counts.sum())
    for c in range(N_CORES):
        g = perms[c].reshape(CHUNKS, P)
        ss = np.full((total, P), -1, np.int64)
        s0 = 0
        for ch in range(CHUNKS):
            kk = int(slot_counts[ch])
            for p in range(P):
                n = g[ch, p]
                if n >= 0 and deg[n] > 0:
                    e = src_by_dst[rptr[n]:rptr[n + 1]]
                    ss[s0:s0 + len(e), p] = e
            s0 += kk
        slot_srcs.append(ss)
    return perms, slot_counts, slot_srcs


def _prepare(feat, W, attn_l, attn_r, bias, src, dst):
    """Preprocess + device program 1, build program-2 input maps."""
    feat = np.asarray(feat, dtype=np.float32)
    W = np.asarray(W, dtype=np.float32)
    attn_l = np.asarray(attn_l, dtype=np.float32).reshape(-1)
    attn_r = np.asarray(attn_r, dtype=np.float32).reshape(-1)
    bias = np.asarray(bias, dtype=np.float32).reshape(-1)
    src = np.asarray(src).astype(np.int64)
    dst = np.asarray(dst).astype(np.int64)

    perms, slot_counts, slot_srcs = _preprocess(src, dst)

    # ---- program 1: T = [ft | el | er] on device (8-way sharded) ----
    if "p1" not in _cache:
        _cache["p1"] = _build_program1()
    nc1 = _cache["p1"]

    featT_pad = np.zeros((D, N_CORES * T1_GRID), np.float32)
    featT_pad[:, :N_NODES] = feat.T
    wl = W @ attn_l
    wr = W @ attn_r
    wall = np.concatenate([W, wl[:, None], wr[:, None]],
                          axis=1).astype(np.float32)
    in_maps1 = []
    for c in range(N_CORES):
        in_maps1.append({
            "featT": np.ascontiguousarray(
                featT_pad[:, c * T1_GRID:(c + 1) * T1_GRID]),
            "wall": np.ascontiguousarray(wall),
        })
    res1 = run_bass_via_pjrt(nc1, in_maps1, N_CORES)
    T_full = np.concatenate([r["tout2"].T for r in res1], axis=0)[:N_NODES]
    # T_full: [N_NODES, 66] = [ft(64) | el | er]

    # ---- host: index-gather tables into per-core streams ----
    ftq = np.zeros((N_NODES + 1, WCOL), np.float32)
    ftq[:N_NODES, 0:D] = T_full[:, 0:D]
    ftq[:N_NODES, D:D + 2] = 1.0
    ftq = ftq.astype(BF16)
    el_tab = np.full(N_NODES + 1, -6e4, np.float32)
    el_tab[:N_NODES] = T_full[:, D]
    er_tab = np.zeros(N_NODES + 1, np.float32)
    er_tab[:N_NODES] = T_full[:, D + 1]
    fres_tab = np.zeros((N_NODES + 1, D), np.float32)
    fres_tab[:N_NODES] = feat + bias
    fres_tab = fres_tab.astype(BF16)
    idn = np.eye(P, dtype=np.float32).astype(BF16)

    stot = int(slot_counts.sum())
    in_maps2 = []
    for c in range(N_CORES):
        ss = slot_srcs[c]                          # [stot, P], -1 pads
        ssx = np.where(ss < 0, N_NODES, ss)
        rows = np.ascontiguousarray(
            ftq[ssx].transpose(1, 0, 2)).reshape(P, stot * WCOL)
        el_g = el_tab[ssx].T                       # [P, stot]
        el2 = np.repeat(el_g, 2, axis=1).astype(np.float16)
        gw = np.where(perms[c] < 0, N_NODES, perms[c])
        er_row = er_tab[gw].reshape(CHUNKS, P)     # [CHUNKS, P]
        er_g = np.repeat(er_row.T, np.asarray(slot_counts, np.int64),
                         axis=1)                   # [P, stot]
        er2 = np.repeat(er_g, 2, axis=1).astype(np.float16)
        in_maps2.append({
            "rows": rows,
            "el2": np.ascontiguousarray(el2),
            "er2": np.ascontiguousarray(er2),
            "idn": np.ascontiguousarray(idn),
            "fres": np.ascontiguousarray(fres_tab[gw].reshape(CHUNKS, P, D)),
        })
    return perms, slot_counts, in_maps2


def kernel(feat, W, attn_l, attn_r, bias, src, dst):
    perms, slot_counts, in_maps2 = _prepare(feat, W, attn_l, attn_r,
                                            bias, src, dst)
    key2 = ("p2", tuple(int(x) for x in slot_counts))
    if key2 not in _cache:
        _cache[key2] = _build_program2(slot_counts)
    res2 = run_bass_via_pjrt(_cache[key2], in_maps2, N_CORES)

    # ---- unshard ----
    rst = np.zeros((N_NODES, D), np.float32)
    for c in range(N_CORES):
        o = res2[c]["out"].reshape(GRID, D).astype(np.float32)
        g = perms[c]
        mask = g >= 0
        rst[g[mask]] = o[mask]
    return rst.reshape(N_NODES, 1, D)


# revision 25
# speedup vs baseline: 1.0163x; 1.0163x over previous
"""Trainium2 Bass kernel for CAGNN (GAT-style) message passing, 8 NeuronCores.

Strategy (edge-parallel, dst-sharded, zero collectives, identity-PE):
  - Each core owns 12,500 destination nodes (1/8 slice). Host sorts each
    core's nodes by in-degree and lays each node's incoming edges in a
    [128-node chunk x slot] grid (common slot profile across cores so all
    8 cores run one SPMD program).
  - Device program 1 (8-way sharded): T = [feat @ W | el | er] where
    el = ft . attn_l, er = ft . attn_r (el = feat @ (W @ attn_l)).
  - Host gathers per-slot [ft | 1 | 1] rows (bf16) plus pair-duplicated
    el/er streams — index copies only, no arithmetic.
  - Device program 2: x2 = exp(leaky_relu(el2 + er2)) computed in a few
    wide ops. Per chunk ONE wide DVE tensor_tensor scales all K slots:
    fw = rows * x_broadcast (the pair-duplicated x layout makes the
    broadcast AP's innermost dim [1,2] so the DVE runs in its fast mode),
    then TensorE accumulates PSUM += I.T @ fw_k per slot (identity
    stationary weights: PE is a pure PSUM accumulator, ~40ns/slot).
    Epilogue in 4-chunk windows: batched max/reciprocal on the PSUM
    denominator column, per-chunk ACT scaled PSUM->SBUF copy, one batched
    residual add, one batched output DMA.
  - Softmax max-subtraction is skipped: e is O(10), exp() is safe in f32,
    and a = exp(e)/sum(exp(e)) is mathematically identical.
"""
import sys

sys.path.insert(0, "/opt/trn_rl_repo")

import numpy as np
import ml_dtypes
from bass_rust import AP
import concourse.bass as bass
import concourse.tile as tile
from concourse import bacc, mybir
from concourse.bass2jax import run_bass_via_pjrt

P = 128
N_NODES = 100000
N_EDGES = 1600000
D = 64
WCOL = D + 2                                  # [ft(64) | 1 | 1]
N_CORES = 8
NODES_PER_CORE = N_NODES // N_CORES           # 12500
CHUNKS = (NODES_PER_CORE + P - 1) // P        # 98
GRID = CHUNKS * P                             # 12544 (44 pad)
T1_TILES = CHUNKS
T1_GRID = T1_TILES * P
NEG_SLOPE = 0.2
WIN = 4                                       # epilogue window (chunks)
BF16 = ml_dtypes.bfloat16

_cache = {}


def _build_program1():
    """T-build: tout2[66, n] = [ft | el | er].T for the core's node slice.

    One matmul per 128-node tile with a FIXED stationary [W | wl | wr]
    (loaded once); 512-col PSUM banks drained by ACT in 4-tile batches.
    """
    nc = bacc.Bacc("TRN2", target_bir_lowering=False, debug=False,
                   num_devices=N_CORES)
    featT = nc.dram_tensor("featT", [D, T1_GRID], mybir.dt.float32,
                           kind="ExternalInput")
    wall = nc.dram_tensor("wall", [D, D + 2], mybir.dt.float32,
                          kind="ExternalInput")
    tout2 = nc.dram_tensor("tout2", [D + 2, T1_GRID], mybir.dt.float32,
                           kind="ExternalOutput")
    with tile.TileContext(nc) as tc:
        with (tc.tile_pool(name="sb", bufs=3) as sb,
              tc.tile_pool(name="ob", bufs=3) as ob,
              tc.tile_pool(name="ps", bufs=4, space="PSUM") as ps,
              tc.tile_pool(name="pers", bufs=1) as pers):
            w_t = pers.tile([D, D + 2], mybir.dt.float32)
            nc.sync.dma_start(w_t[:], wall[:, :])
            B = 4                               # tiles per PSUM bank
            for t0 in range(0, T1_TILES, B):
                nb = min(B, T1_TILES - t0)
                ftT = sb.tile([D, B * P], mybir.dt.float32, tag="ftT")
                nc.sync.dma_start(ftT[:, 0:nb * P],
                                  featT[:, t0 * P:(t0 + nb) * P])
                psum = ps.tile([D + 2, 512], mybir.dt.float32, space="PSUM",
                               tag="t")
                nc.tensor.matmul(psum[:, 0:nb * P], lhsT=w_t[:],
                                 rhs=ftT[:, 0:nb * P],
                                 start=True, stop=True)
                row = ob.tile([D + 2, B * P], mybir.dt.float32, tag="row")
                nc.scalar.copy(row[:, 0:nb * P], psum[:, 0:nb * P])
                nc.sync.dma_start(tout2[:, t0 * P:(t0 + nb) * P],
                                  row[:, 0:nb * P])
    nc.finalize()
    return nc


def _build_program2(slot_counts):
    """Identity-PE aggregation pass. slot_counts[ch] = slots in chunk ch."""
    slot_counts = [int(s) for s in slot_counts]
    stot = sum(slot_counts)
    s2 = 2 * stot
    nc = bacc.Bacc("TRN2", target_bir_lowering=False, debug=False,
                   num_devices=N_CORES)
    rows = nc.dram_tensor("rows", [P, stot * WCOL], mybir.dt.bfloat16,
                          kind="ExternalInput")
    el2 = nc.dram_tensor("el2", [P, s2], mybir.dt.float16,
                         kind="ExternalInput")
    er2 = nc.dram_tensor("er2", [P, s2], mybir.dt.float16,
                         kind="ExternalInput")
    idn = nc.dram_tensor("idn", [P, P], mybir.dt.bfloat16,
                         kind="ExternalInput")
    fres = nc.dram_tensor("fres", [CHUNKS, P, D], mybir.dt.bfloat16,
                          kind="ExternalInput")
    out = nc.dram_tensor("out", [CHUNKS, P, D], mybir.dt.bfloat16,
                         kind="ExternalOutput")
    # chunk -> slot offset; x-tile boundaries aligned to chunk starts
    s_off = [0]
    for kk in slot_counts:
        s_off.append(s_off[-1] + kk)
    XT = 512                       # x-tile max width (slots)
    RG = WIN                       # rows-DMA group (chunks)
    rbounds = [0, 1, 2, 4, 8]
    while rbounds[-1] < CHUNKS:
        rbounds.append(min(rbounds[-1] + RG, CHUNKS))
    # x-tiles = unions of rows-groups (staircase targets, then XT)
    xtiles = []
    targets = [64, 192, 448]
    ti = 0
    i = 0
    while i < len(rbounds) - 1:
        tgt = targets[ti] if ti < len(targets) else XT
        ti += 1
        j = i + 1
        while (j < len(rbounds) - 1
               and s_off[rbounds[j]] - s_off[rbounds[i]] < tgt):
            j += 1
        xtiles.append((rbounds[i], rbounds[j]))
        i = j
    xt_max = max(s_off[b] - s_off[a] for a, b in xtiles)

    with tile.TileContext(nc) as tc:
        with (tc.tile_pool(name="rp", bufs=2) as rp,
              tc.tile_pool(name="fp", bufs=2) as fp,
              tc.tile_pool(name="xp", bufs=4) as xp,
              tc.tile_pool(name="ep", bufs=3) as ep,
              tc.tile_pool(name="wp", bufs=3) as wp,
              tc.tile_pool(name="ps", bufs=8, space="PSUM") as ps,
              tc.tile_pool(name="sp", bufs=3) as sp,
              tc.tile_pool(name="pers", bufs=1) as pers):
            i_t = pers.tile([P, P], mybir.dt.bfloat16)
            nc.sync.dma_start(i_t[:], idn[:, :])
            el_t = pers.tile([P, s2], mybir.dt.float16)
            er_t = pers.tile([P, s2], mybir.dt.float16)
            hd = min(256, s2)
            nc.sync.dma_start(el_t[:, 0:hd], el2[:, 0:hd])
            nc.sync.dma_start(er_t[:, 0:hd], er2[:, 0:hd])
            h = s2 // 2
            nc.sync.dma_start(el_t[:, hd:h], el2[:, hd:h])
            nc.scalar.dma_start(el_t[:, h:s2], el2[:, h:s2])
            nc.sync.dma_start(er_t[:, hd:h], er2[:, hd:h])
            nc.scalar.dma_start(er_t[:, h:s2], er2[:, h:s2])
            fres_t = pers.tile([P, CHUNKS * D], mybir.dt.bfloat16)
            fsrc_all = AP(fres[0].tensor, 0, [[D, P], [P * D, CHUNKS], [1, D]])
            nc.scalar.dma_start(fres_t[:], fsrc_all)

            cur_e = [None, -1]       # (tile, xtile idx)

            def emit_xtile(ti):
                """e = leaky_relu(el2 + er2) for the tile's slot range."""
                clo, chi = xtiles[ti]
                t0, t1 = 2 * s_off[clo], 2 * s_off[chi]
                tw = t1 - t0
                e_s = ep.tile([P, 2 * xt_max], mybir.dt.float16, tag="e")
                nc.vector.tensor_add(e_s[:, 0:tw], el_t[:, t0:t1],
                                     er_t[:, t0:t1])
                nc.vector.scalar_tensor_tensor(
                    out=e_s[:, 0:tw], in0=e_s[:, 0:tw], scalar=NEG_SLOPE,
                    in1=e_s[:, 0:tw],
                    op0=mybir.AluOpType.mult, op1=mybir.AluOpType.max)
                x2_s = xp.tile([P, 2 * xt_max], mybir.dt.bfloat16, tag="x2")
                nc.scalar.activation(x2_s[:, 0:tw], e_s[:, 0:tw],
                                     mybir.ActivationFunctionType.Exp)
                return x2_s

            def epilogue(w0, w_psums, rec_w):
                """Batched scale+residual for chunks w0..w0+nw-1."""
                nw = len(w_psums)
                t_w = sp.tile([P, WIN * D], mybir.dt.bfloat16, tag="t")
                for i, psum in enumerate(w_psums):
                    nc.scalar.activation(t_w[:, i * D:(i + 1) * D],
                                         psum[:, 0:D],
                                         mybir.ActivationFunctionType.Copy,
                                         scale=rec_w[:, i:i + 1])
                o_w = sp.tile([P, WIN * D], mybir.dt.bfloat16, tag="o")
                nc.vector.tensor_add(o_w[:, 0:nw * D], t_w[:, 0:nw * D],
                                     fres_t[:, w0 * D:(w0 + nw) * D])
                odst = AP(out[0].tensor, w0 * P * D,
                          [[D, P], [P * D, nw], [1, D]])
                nc.gpsimd.dma_start(odst, o_w[:, 0:nw * D])

            # lag-1 window pipeline: emit window w's epilogue after window
            # w+1's multiplies so no engine waits on the PE in-line
            ready = None
            pend_w0 = None
            pend_psums = []
            den_w = None
            pend_e = None
            next_xt = 0
            rg_max = max(s_off[b] - s_off[a]
                         for a, b in zip(rbounds, rbounds[1:]))
            rstarts = set(rbounds[:-1])
            rg_bound = dict(zip(rbounds, rbounds[1:]))
            rg_tile = None
            fw_g = None
            rg_lo = -1
            for ch in range(CHUNKS):
                kk = slot_counts[ch]
                s0 = s_off[ch]
                while next_xt < len(xtiles) and xtiles[next_xt][0] <= ch + 6:
                    nxt = [emit_xtile(next_xt), next_xt]
                    if next_xt == 0:
                        cur_e = nxt
                    else:
                        pend_e = nxt
                    next_xt += 1
                if cur_e[1] >= 0 and ch >= xtiles[cur_e[1]][1]:
                    cur_e = pend_e
                if ch in rstarts:
                    glo, ghi = ch, rg_bound[ch]
                    gk = s_off[ghi] - s_off[glo]
                    gw = gk * WCOL
                    rg_tile = rp.tile([P, rg_max * WCOL],
                                      mybir.dt.bfloat16, tag="rows")
                    nc.sync.dma_start(
                        rg_tile[:, 0:gw],
                        rows[:, s_off[glo] * WCOL:s_off[ghi] * WCOL])
                    rg_lo = glo
                    # one wide fw = rows * x_broadcast for the whole group
                    x2g = cur_e[0]
                    xlo = xtiles[cur_e[1]][0]
                    goff = 2 * (s_off[glo] - s_off[xlo])
                    fw_g = fp.tile([P, rg_max * WCOL], mybir.dt.bfloat16,
                                   tag="fw")
                    xbg = AP(x2g[:].tensor, goff,
                             [[2 * xt_max, P], [2, gk], [0, WCOL // 2],
                              [1, 2]])
                    nc.vector.tensor_mul(fw_g[:, 0:gw],
                                         rg_tile[:, 0:gw], xbg)
                if pend_w0 is None:
                    pend_w0 = ch
                    den_w = wp.tile([P, WIN], mybir.dt.float32, tag="den")
                foff = (s0 - s_off[rg_lo]) * WCOL
                psum = ps.tile([P, 512], mybir.dt.float32, space="PSUM",
                               tag="acc")
                for k in range(kk):
                    nc.tensor.matmul(
                        psum[:, 0:WCOL], lhsT=i_t[:],
                        rhs=fw_g[:, foff + k * WCOL:foff + (k + 1) * WCOL],
                        start=(k == 0), stop=(k == kk - 1))
                # denominator (2*sum_k x) sits in PSUM col D (ones column)
                nc.scalar.copy(den_w[:, ch - pend_w0:ch - pend_w0 + 1],
                               psum[:, D:D + 1])
                pend_psums.append(psum)
                if len(pend_psums) == WIN or ch == CHUNKS - 1:
                    # rec = 1 / max(den/2, eps): depends only on the exps
                    nw = len(pend_psums)
                    nc.vector.tensor_scalar_max(den_w[:, 0:nw],
                                                den_w[:, 0:nw], 1e-30)
                    rec_w = wp.tile([P, WIN], mybir.dt.float32, tag="rec")
                    nc.vector.reciprocal(rec_w[:, 0:nw], den_w[:, 0:nw])
                    if ready is not None:
                        epilogue(*ready)
                    ready = (pend_w0, pend_psums, rec_w)
                    pend_w0, pend_psums = None, []
            if ready is not None:
                epilogue(*ready)
    nc.finalize()
    return nc


def _preprocess(src, dst):
    """Edge layout: per-core degree-sorted chunk/slot grid, common profile.

    Returns (perm[core][GRID] node-ids with -1 pads, slot_counts[CHUNKS],
    slot_src[core] int64 [total_slots, P] with -1 for pad slots).
    """
    deg = np.bincount(dst, minlength=N_NODES)
    order = np.argsort(dst, kind="stable")
    src_by_dst = src[order]
    rptr = np.zeros(N_NODES + 1, np.int64)
    np.cumsum(deg, out=rptr[1:])

    perms = []
    percore_counts = np.zeros((N_CORES, CHUNKS), np.int64)
    for c in range(N_CORES):
        lo = c * NODES_PER_CORE
        nodes = np.arange(lo, lo + NODES_PER_CORE)
        p = nodes[np.argsort(deg[nodes], kind="stable")]
        grid = np.full(GRID, -1, np.int64)
        grid[GRID - NODES_PER_CORE:] = p          # pads first (low-deg end)
        perms.append(grid)
        g = grid.reshape(CHUNKS, P)
        for ch in range(CHUNKS):
            real = g[ch][g[ch] >= 0]
            percore_counts[c, ch] = deg[real].max() if len(real) else 0
    slot_counts = np.maximum(1, percore_counts.max(axis=0))

    slot_srcs = []
    total = int(slot_counts.sum())
    for c in range(N_CORES):
        g = perms[c].reshape(CHUNKS, P)
        ss = np.full((total, P), -1, np.int64)
        s0 = 0
        for ch in range(CHUNKS):
            kk = int(slot_counts[ch])
            for p in range(P):
                n = g[ch, p]
                if n >= 0 and deg[n] > 0:
                    e = src_by_dst[rptr[n]:rptr[n + 1]]
                    ss[s0:s0 + len(e), p] = e
            s0 += kk
        slot_srcs.append(ss)
    return perms, slot_counts, slot_srcs


def _prepare(feat, W, attn_l, attn_r, bias, src, dst):
    """Preprocess + device program 1, build program-2 input maps."""
    feat = np.asarray(feat, dtype=np.float32)
    W = np.asarray(W, dtype=np.float32)
    attn_l = np.asarray(attn_l, dtype=np.float32).reshape(-1)
    attn_r = np.asarray(attn_r, dtype=np.float32).reshape(-1)
    bias = np.asarray(bias, dtype=np.float32).reshape(-1)
    src = np.asarray(src).astype(np.int64)
    dst = np.asarray(dst).astype(np.int64)

    perms, slot_counts, slot_srcs = _preprocess(src, dst)

    # ---- program 1: T = [ft | el | er] on device (8-way sharded) ----
    if "p1" not in _cache:
        _cache["p1"] = _build_program1()
    nc1 = _cache["p1"]

    featT_pad = np.zeros((D, N_CORES * T1_GRID), np.float32)
    featT_pad[:, :N_NODES] = feat.T
    wl = W @ attn_l
    wr = W @ attn_r
    wall = np.concatenate([W, wl[:, None], wr[:, None]],
                          axis=1).astype(np.float32)
    in_maps1 = []
    for c in range(N_CORES):
        in_maps1.append({
            "featT": np.ascontiguousarray(
                featT_pad[:, c * T1_GRID:(c + 1) * T1_GRID]),
            "wall": np.ascontiguousarray(wall),
        })
    res1 = run_bass_via_pjrt(nc1, in_maps1, N_CORES)
    T_full = np.concatenate([r["tout2"].T for r in res1], axis=0)[:N_NODES]
    # T_full: [N_NODES, 66] = [ft(64) | el | er]

    # ---- host: index-gather tables into per-core streams ----
    ftq = np.zeros((N_NODES + 1, WCOL), np.float32)
    ftq[:N_NODES, 0:D] = T_full[:, 0:D]
    ftq[:N_NODES, D:D + 2] = 1.0
    ftq = ftq.astype(BF16)
    el_tab = np.full(N_NODES + 1, -6e4, np.float32)
    el_tab[:N_NODES] = T_full[:, D]
    er_tab = np.zeros(N_NODES + 1, np.float32)
    er_tab[:N_NODES] = T_full[:, D + 1]
    fres_tab = np.zeros((N_NODES + 1, D), np.float32)
    fres_tab[:N_NODES] = feat + bias
    fres_tab = fres_tab.astype(BF16)
    idn = np.eye(P, dtype=np.float32).astype(BF16)

    stot = int(slot_counts.sum())
    in_maps2 = []
    for c in range(N_CORES):
        ss = slot_srcs[c]                          # [stot, P], -1 pads
        ssx = np.where(ss < 0, N_NODES, ss)
        rows = np.ascontiguousarray(
            ftq[ssx].transpose(1, 0, 2)).reshape(P, stot * WCOL)
        el_g = el_tab[ssx].T                       # [P, stot]
        el2 = np.repeat(el_g, 2, axis=1).astype(np.float16)
        gw = np.where(perms[c] < 0, N_NODES, perms[c])
        er_row = er_tab[gw].reshape(CHUNKS, P)     # [CHUNKS, P]
        er_g = np.repeat(er_row.T, np.asarray(slot_counts, np.int64),
                         axis=1)                   # [P, stot]
        er2 = np.repeat(er_g, 2, axis=1).astype(np.float16)
        in_maps2.append({
            "rows": rows,
            "el2": np.ascontiguousarray(el2),
            "er2": np.ascontiguousarray(er2),
            "idn": np.ascontiguousarray(idn),
            "fres": np.ascontiguousarray(fres_tab[gw].reshape(CHUNKS, P, D)),
        })
    return perms, slot_counts, in_maps2


def kernel(feat, W, attn_l, attn_r, bias, src, dst):
    perms, slot_counts, in_maps2 = _prepare(feat, W, attn_l, attn_r,
                                            bias, src, dst)
    key2 = ("p2", tuple(int(x) for x in slot_counts))
    if key2 not in _cache:
        _cache[key2] = _build_program2(slot_counts)
    res2 = run_bass_via_pjrt(_cache[key2], in_maps2, N_CORES)

    # ---- unshard ----
    rst = np.zeros((N_NODES, D), np.float32)
    for c in range(N_CORES):
        o = res2[c]["out"].reshape(GRID, D).astype(np.float32)
        g = perms[c]
        mask = g >= 0
        rst[g[mask]] = o[mask]
    return rst.reshape(N_NODES, 1, D)


# revision 26
# speedup vs baseline: 1.1720x; 1.1531x over previous
"""Trainium2 Bass kernel for CAGNN (GAT-style) message passing, 8 NeuronCores.

Strategy (edge-parallel, dst-sharded, zero collectives, identity-PE):
  - Each core owns 12,500 destination nodes (1/8 slice). Host sorts each
    core's nodes by in-degree and lays each node's incoming edges in a
    [128-node chunk x slot] grid (common slot profile across cores so all
    8 cores run one SPMD program).
  - Device program 1 (8-way sharded): T = [feat @ W | el | er] where
    el = ft . attn_l, er = ft . attn_r (el = feat @ (W @ attn_l)).
  - Host gathers per-slot [ft | 1 | 1] rows (bf16) plus pair-duplicated
    el/er streams — index copies only, no arithmetic.
  - Device program 2: x2 = exp(leaky_relu(el2 + er2)) computed in a few
    wide ops. Per chunk ONE wide DVE tensor_tensor scales all K slots:
    fw = rows * x_broadcast (the pair-duplicated x layout makes the
    broadcast AP's innermost dim [1,2] so the DVE runs in its fast mode),
    then TensorE accumulates PSUM += I.T @ fw_k per slot (identity
    stationary weights: PE is a pure PSUM accumulator, ~40ns/slot).
    Epilogue in 4-chunk windows: batched max/reciprocal on the PSUM
    denominator column, per-chunk ACT scaled PSUM->SBUF copy, one batched
    residual add, one batched output DMA.
  - Softmax max-subtraction is skipped: e is O(10), exp() is safe in f32,
    and a = exp(e)/sum(exp(e)) is mathematically identical.
"""
import sys

sys.path.insert(0, "/opt/trn_rl_repo")

import numpy as np
import ml_dtypes
from bass_rust import AP
import concourse.bass as bass
import concourse.tile as tile
from concourse import bacc, mybir
from concourse.bass2jax import run_bass_via_pjrt

P = 128
N_NODES = 100000
N_EDGES = 1600000
D = 64
WCOL = D + 2                                  # [ft(64) | 1 | 1]
N_CORES = 8
NODES_PER_CORE = N_NODES // N_CORES           # 12500
CHUNKS = (NODES_PER_CORE + P - 1) // P        # 98
GRID = CHUNKS * P                             # 12544 (44 pad)
T1_TILES = CHUNKS
T1_GRID = T1_TILES * P
NEG_SLOPE = 0.2
WIN = 4                                       # epilogue window (chunks)
BF16 = ml_dtypes.bfloat16

_cache = {}


def _build_program1():
    """T-build: tout2[66, n] = [ft | el | er].T for the core's node slice.

    One matmul per 128-node tile with a FIXED stationary [W | wl | wr]
    (loaded once); 512-col PSUM banks drained by ACT in 4-tile batches.
    """
    nc = bacc.Bacc("TRN2", target_bir_lowering=False, debug=False,
                   num_devices=N_CORES)
    featT = nc.dram_tensor("featT", [D, T1_GRID], mybir.dt.float32,
                           kind="ExternalInput")
    wall = nc.dram_tensor("wall", [D, D + 2], mybir.dt.float32,
                          kind="ExternalInput")
    tout2 = nc.dram_tensor("tout2", [D + 2, T1_GRID], mybir.dt.float32,
                           kind="ExternalOutput")
    with tile.TileContext(nc) as tc:
        with (tc.tile_pool(name="sb", bufs=3) as sb,
              tc.tile_pool(name="ob", bufs=3) as ob,
              tc.tile_pool(name="ps", bufs=4, space="PSUM") as ps,
              tc.tile_pool(name="pers", bufs=1) as pers):
            w_t = pers.tile([D, D + 2], mybir.dt.float32)
            nc.sync.dma_start(w_t[:], wall[:, :])
            B = 4                               # tiles per PSUM bank
            for t0 in range(0, T1_TILES, B):
                nb = min(B, T1_TILES - t0)
                ftT = sb.tile([D, B * P], mybir.dt.float32, tag="ftT")
                nc.sync.dma_start(ftT[:, 0:nb * P],
                                  featT[:, t0 * P:(t0 + nb) * P])
                psum = ps.tile([D + 2, 512], mybir.dt.float32, space="PSUM",
                               tag="t")
                nc.tensor.matmul(psum[:, 0:nb * P], lhsT=w_t[:],
                                 rhs=ftT[:, 0:nb * P],
                                 start=True, stop=True)
                row = ob.tile([D + 2, B * P], mybir.dt.float32, tag="row")
                nc.scalar.copy(row[:, 0:nb * P], psum[:, 0:nb * P])
                nc.sync.dma_start(tout2[:, t0 * P:(t0 + nb) * P],
                                  row[:, 0:nb * P])
    nc.finalize()
    return nc


def _build_program2(slot_counts):
    """Identity-PE aggregation pass. slot_counts[ch] = slots in chunk ch."""
    slot_counts = [int(s) for s in slot_counts]
    stot = sum(slot_counts)
    s2 = 2 * stot
    nc = bacc.Bacc("TRN2", target_bir_lowering=False, debug=False,
                   num_devices=N_CORES)
    rows = nc.dram_tensor("rows", [P, stot * WCOL], mybir.dt.bfloat16,
                          kind="ExternalInput")
    el2 = nc.dram_tensor("el2", [P, s2], mybir.dt.float16,
                         kind="ExternalInput")
    er2 = nc.dram_tensor("er2", [P, s2], mybir.dt.float16,
                         kind="ExternalInput")
    idn = nc.dram_tensor("idn", [P, P], mybir.dt.bfloat16,
                         kind="ExternalInput")
    fres = nc.dram_tensor("fres", [CHUNKS, P, D], mybir.dt.bfloat16,
                          kind="ExternalInput")
    out = nc.dram_tensor("out", [CHUNKS, P, D], mybir.dt.bfloat16,
                         kind="ExternalOutput")
    # chunk -> slot offset; x-tile boundaries aligned to chunk starts
    s_off = [0]
    for kk in slot_counts:
        s_off.append(s_off[-1] + kk)
    XT = 512                       # x-tile max width (slots)
    RG = WIN                       # rows-DMA group (chunks)
    rbounds = [0, 1, 2, 4, 8]
    while rbounds[-1] < CHUNKS:
        rbounds.append(min(rbounds[-1] + RG, CHUNKS))
    # x-tiles = unions of rows-groups (staircase targets, then XT)
    xtiles = []
    targets = [64, 192, 448]
    ti = 0
    i = 0
    while i < len(rbounds) - 1:
        tgt = targets[ti] if ti < len(targets) else XT
        ti += 1
        j = i + 1
        while (j < len(rbounds) - 1
               and s_off[rbounds[j]] - s_off[rbounds[i]] < tgt):
            j += 1
        xtiles.append((rbounds[i], rbounds[j]))
        i = j
    xt_max = max(s_off[b] - s_off[a] for a, b in xtiles)

    with tile.TileContext(nc) as tc:
        with (tc.tile_pool(name="rp", bufs=3) as rp,
              tc.tile_pool(name="fp", bufs=3) as fp,
              tc.tile_pool(name="xp", bufs=4) as xp,
              tc.tile_pool(name="ep", bufs=3) as ep,
              tc.tile_pool(name="wp", bufs=3) as wp,
              tc.tile_pool(name="ps", bufs=8, space="PSUM") as ps,
              tc.tile_pool(name="sp", bufs=3) as sp,
              tc.tile_pool(name="pers", bufs=1) as pers):
            i_t = pers.tile([P, P], mybir.dt.bfloat16)
            nc.sync.dma_start(i_t[:], idn[:, :])
            el_t = pers.tile([P, s2], mybir.dt.float16)
            er_t = pers.tile([P, s2], mybir.dt.float16)
            hd = min(256, s2)
            nc.sync.dma_start(el_t[:, 0:hd], el2[:, 0:hd])
            nc.sync.dma_start(er_t[:, 0:hd], er2[:, 0:hd])
            h = s2 // 2
            nc.sync.dma_start(el_t[:, hd:h], el2[:, hd:h])
            nc.scalar.dma_start(el_t[:, h:s2], el2[:, h:s2])
            nc.sync.dma_start(er_t[:, hd:h], er2[:, hd:h])
            nc.scalar.dma_start(er_t[:, h:s2], er2[:, h:s2])
            fres_t = pers.tile([P, CHUNKS * D], mybir.dt.bfloat16)
            fsrc_all = AP(fres[0].tensor, 0, [[D, P], [P * D, CHUNKS], [1, D]])
            nc.scalar.dma_start(fres_t[:], fsrc_all)

            cur_e = [None, -1]       # (tile, xtile idx)

            def emit_xtile(ti):
                """e = leaky_relu(el2 + er2) for the tile's slot range."""
                clo, chi = xtiles[ti]
                t0, t1 = 2 * s_off[clo], 2 * s_off[chi]
                tw = t1 - t0
                e_s = ep.tile([P, 2 * xt_max], mybir.dt.float16, tag="e")
                nc.vector.tensor_add(e_s[:, 0:tw], el_t[:, t0:t1],
                                     er_t[:, t0:t1])
                nc.vector.scalar_tensor_tensor(
                    out=e_s[:, 0:tw], in0=e_s[:, 0:tw], scalar=NEG_SLOPE,
                    in1=e_s[:, 0:tw],
                    op0=mybir.AluOpType.mult, op1=mybir.AluOpType.max)
                x2_s = xp.tile([P, 2 * xt_max], mybir.dt.bfloat16, tag="x2")
                nc.scalar.activation(x2_s[:, 0:tw], e_s[:, 0:tw],
                                     mybir.ActivationFunctionType.Exp)
                return x2_s

            def epilogue(w0, w_psums, rec_w):
                """Batched scale+residual for chunks w0..w0+nw-1."""
                nw = len(w_psums)
                t_w = sp.tile([P, WIN * D], mybir.dt.bfloat16, tag="t")
                for i, psum in enumerate(w_psums):
                    nc.scalar.activation(t_w[:, i * D:(i + 1) * D],
                                         psum[:, 0:D],
                                         mybir.ActivationFunctionType.Copy,
                                         scale=rec_w[:, i:i + 1])
                o_w = sp.tile([P, WIN * D], mybir.dt.bfloat16, tag="o")
                nc.vector.tensor_add(o_w[:, 0:nw * D], t_w[:, 0:nw * D],
                                     fres_t[:, w0 * D:(w0 + nw) * D])
                odst = AP(out[0].tensor, w0 * P * D,
                          [[D, P], [P * D, nw], [1, D]])
                nc.gpsimd.dma_start(odst, o_w[:, 0:nw * D])

            # lag-1 window pipeline: emit window w's epilogue after window
            # w+1's multiplies so no engine waits on the PE in-line
            ready = None
            pend_w0 = None
            pend_psums = []
            den_w = None
            pend_e = None
            next_xt = 0
            rg_max = max(s_off[b] - s_off[a]
                         for a, b in zip(rbounds, rbounds[1:]))
            rstarts = set(rbounds[:-1])
            rg_bound = dict(zip(rbounds, rbounds[1:]))
            rg_tile = None
            fw_g = None
            rg_lo = -1
            for ch in range(CHUNKS):
                kk = slot_counts[ch]
                s0 = s_off[ch]
                while next_xt < len(xtiles) and xtiles[next_xt][0] <= ch + 6:
                    nxt = [emit_xtile(next_xt), next_xt]
                    if next_xt == 0:
                        cur_e = nxt
                    else:
                        pend_e = nxt
                    next_xt += 1
                if cur_e[1] >= 0 and ch >= xtiles[cur_e[1]][1]:
                    cur_e = pend_e
                if ch in rstarts:
                    glo, ghi = ch, rg_bound[ch]
                    gk = s_off[ghi] - s_off[glo]
                    gw = gk * WCOL
                    rg_tile = rp.tile([P, rg_max * WCOL],
                                      mybir.dt.bfloat16, tag="rows")
                    nc.sync.dma_start(
                        rg_tile[:, 0:gw],
                        rows[:, s_off[glo] * WCOL:s_off[ghi] * WCOL])
                    rg_lo = glo
                    # one wide fw = rows * x_broadcast for the whole group
                    x2g = cur_e[0]
                    xlo = xtiles[cur_e[1]][0]
                    goff = 2 * (s_off[glo] - s_off[xlo])
                    fw_g = fp.tile([P, rg_max * WCOL], mybir.dt.bfloat16,
                                   tag="fw")
                    xbg = AP(x2g[:].tensor, goff,
                             [[2 * xt_max, P], [2, gk], [0, WCOL // 2],
                              [1, 2]])
                    nc.vector.tensor_mul(fw_g[:, 0:gw],
                                         rg_tile[:, 0:gw], xbg)
                if pend_w0 is None:
                    pend_w0 = ch
                    den_w = wp.tile([P, WIN], mybir.dt.float32, tag="den")
                foff = (s0 - s_off[rg_lo]) * WCOL
                psum = ps.tile([P, 512], mybir.dt.float32, space="PSUM",
                               tag="acc")
                for k in range(kk):
                    nc.tensor.matmul(
                        psum[:, 0:WCOL], lhsT=i_t[:],
                        rhs=fw_g[:, foff + k * WCOL:foff + (k + 1) * WCOL],
                        start=(k == 0), stop=(k == kk - 1))
                # denominator (2*sum_k x) sits in PSUM col D (ones column)
                nc.scalar.copy(den_w[:, ch - pend_w0:ch - pend_w0 + 1],
                               psum[:, D:D + 1])
                pend_psums.append(psum)
                if len(pend_psums) == WIN or ch == CHUNKS - 1:
                    # rec = 1 / max(den/2, eps): depends only on the exps
                    nw = len(pend_psums)
                    nc.vector.tensor_scalar_max(den_w[:, 0:nw],
                                                den_w[:, 0:nw], 1e-30)
                    rec_w = wp.tile([P, WIN], mybir.dt.float32, tag="rec")
                    nc.vector.reciprocal(rec_w[:, 0:nw], den_w[:, 0:nw])
                    if ready is not None:
                        epilogue(*ready)
                    ready = (pend_w0, pend_psums, rec_w)
                    pend_w0, pend_psums = None, []
            if ready is not None:
                epilogue(*ready)
    nc.finalize()
    return nc


def _preprocess(src, dst):
    """Edge layout: per-core degree-sorted chunk/slot grid, common profile.

    Returns (perm[core][GRID] node-ids with -1 pads, slot_counts[CHUNKS],
    slot_src[core] int64 [total_slots, P] with -1 for pad slots).
    """
    deg = np.bincount(dst, minlength=N_NODES)
    order = np.argsort(dst, kind="stable")
    src_by_dst = src[order]
    rptr = np.zeros(N_NODES + 1, np.int64)
    np.cumsum(deg, out=rptr[1:])

    perms = []
    percore_counts = np.zeros((N_CORES, CHUNKS), np.int64)
    for c in range(N_CORES):
        lo = c * NODES_PER_CORE
        nodes = np.arange(lo, lo + NODES_PER_CORE)
        p = nodes[np.argsort(deg[nodes], kind="stable")]
        grid = np.full(GRID, -1, np.int64)
        grid[GRID - NODES_PER_CORE:] = p          # pads first (low-deg end)
        perms.append(grid)
        g = grid.reshape(CHUNKS, P)
        for ch in range(CHUNKS):
            real = g[ch][g[ch] >= 0]
            percore_counts[c, ch] = deg[real].max() if len(real) else 0
    slot_counts = np.maximum(1, percore_counts.max(axis=0))

    slot_srcs = []
    total = int(slot_counts.sum())
    for c in range(N_CORES):
        g = perms[c].reshape(CHUNKS, P)
        ss = np.full((total, P), -1, np.int64)
        s0 = 0
        for ch in range(CHUNKS):
            kk = int(slot_counts[ch])
            for p in range(P):
                n = g[ch, p]
                if n >= 0 and deg[n] > 0:
                    e = src_by_dst[rptr[n]:rptr[n + 1]]
                    ss[s0:s0 + len(e), p] = e
            s0 += kk
        slot_srcs.append(ss)
    return perms, slot_counts, slot_srcs


def _prepare(feat, W, attn_l, attn_r, bias, src, dst):
    """Preprocess + device program 1, build program-2 input maps."""
    feat = np.asarray(feat, dtype=np.float32)
    W = np.asarray(W, dtype=np.float32)
    attn_l = np.asarray(attn_l, dtype=np.float32).reshape(-1)
    attn_r = np.asarray(attn_r, dtype=np.float32).reshape(-1)
    bias = np.asarray(bias, dtype=np.float32).reshape(-1)
    src = np.asarray(src).astype(np.int64)
    dst = np.asarray(dst).astype(np.int64)

    perms, slot_counts, slot_srcs = _preprocess(src, dst)

    # ---- program 1: T = [ft | el | er] on device (8-way sharded) ----
    if "p1" not in _cache:
        _cache["p1"] = _build_program1()
    nc1 = _cache["p1"]

    featT_pad = np.zeros((D, N_CORES * T1_GRID), np.float32)
    featT_pad[:, :N_NODES] = feat.T
    wl = W @ attn_l
    wr = W @ attn_r
    wall = np.concatenate([W, wl[:, None], wr[:, None]],
                          axis=1).astype(np.float32)
    in_maps1 = []
    for c in range(N_CORES):
        in_maps1.append({
            "featT": np.ascontiguousarray(
                featT_pad[:, c * T1_GRID:(c + 1) * T1_GRID]),
            "wall": np.ascontiguousarray(wall),
        })
    res1 = run_bass_via_pjrt(nc1, in_maps1, N_CORES)
    T_full = np.concatenate([r["tout2"].T for r in res1], axis=0)[:N_NODES]
    # T_full: [N_NODES, 66] = [ft(64) | el | er]

    # ---- host: index-gather tables into per-core streams ----
    ftq = np.zeros((N_NODES + 1, WCOL), np.float32)
    ftq[:N_NODES, 0:D] = T_full[:, 0:D]
    ftq[:N_NODES, D:D + 2] = 1.0
    ftq = ftq.astype(BF16)
    el_tab = np.full(N_NODES + 1, -6e4, np.float32)
    el_tab[:N_NODES] = T_full[:, D]
    er_tab = np.zeros(N_NODES + 1, np.float32)
    er_tab[:N_NODES] = T_full[:, D + 1]
    fres_tab = np.zeros((N_NODES + 1, D), np.float32)
    fres_tab[:N_NODES] = feat + bias
    fres_tab = fres_tab.astype(BF16)
    idn = np.eye(P, dtype=np.float32).astype(BF16)

    stot = int(slot_counts.sum())
    in_maps2 = []
    for c in range(N_CORES):
        ss = slot_srcs[c]                          # [stot, P], -1 pads
        ssx = np.where(ss < 0, N_NODES, ss)
        rows = np.ascontiguousarray(
            ftq[ssx].transpose(1, 0, 2)).reshape(P, stot * WCOL)
        el_g = el_tab[ssx].T                       # [P, stot]
        el2 = np.repeat(el_g, 2, axis=1).astype(np.float16)
        gw = np.where(perms[c] < 0, N_NODES, perms[c])
        er_row = er_tab[gw].reshape(CHUNKS, P)     # [CHUNKS, P]
        er_g = np.repeat(er_row.T, np.asarray(slot_counts, np.int64),
                         axis=1)                   # [P, stot]
        er2 = np.repeat(er_g, 2, axis=1).astype(np.float16)
        in_maps2.append({
            "rows": rows,
            "el2": np.ascontiguousarray(el2),
            "er2": np.ascontiguousarray(er2),
            "idn": np.ascontiguousarray(idn),
            "fres": np.ascontiguousarray(fres_tab[gw].reshape(CHUNKS, P, D)),
        })
    return perms, slot_counts, in_maps2


def kernel(feat, W, attn_l, attn_r, bias, src, dst):
    perms, slot_counts, in_maps2 = _prepare(feat, W, attn_l, attn_r,
                                            bias, src, dst)
    key2 = ("p2", tuple(int(x) for x in slot_counts))
    if key2 not in _cache:
        _cache[key2] = _build_program2(slot_counts)
    res2 = run_bass_via_pjrt(_cache[key2], in_maps2, N_CORES)

    # ---- unshard ----
    rst = np.zeros((N_NODES, D), np.float32)
    for c in range(N_CORES):
        o = res2[c]["out"].reshape(GRID, D).astype(np.float32)
        g = perms[c]
        mask = g >= 0
        rst[g[mask]] = o[mask]
    return rst.reshape(N_NODES, 1, D)


# revision 27
# speedup vs baseline: 1.1818x; 1.0084x over previous
"""Trainium2 Bass kernel for CAGNN (GAT-style) message passing, 8 NeuronCores.

Strategy (edge-parallel, dst-sharded, zero collectives, identity-PE):
  - Each core owns 12,500 destination nodes (1/8 slice). Host sorts each
    core's nodes by in-degree and lays each node's incoming edges in a
    [128-node chunk x slot] grid (common slot profile across cores so all
    8 cores run one SPMD program).
  - Device program 1 (8-way sharded): T = [feat @ W | el | er] where
    el = ft . attn_l, er = ft . attn_r (el = feat @ (W @ attn_l)).
  - Host gathers per-slot [ft | 1 | 1] rows (bf16) plus pair-duplicated
    el/er streams — index copies only, no arithmetic.
  - Device program 2: x2 = exp(leaky_relu(el2 + er2)) computed in a few
    wide ops. Per chunk ONE wide DVE tensor_tensor scales all K slots:
    fw = rows * x_broadcast (the pair-duplicated x layout makes the
    broadcast AP's innermost dim [1,2] so the DVE runs in its fast mode),
    then TensorE accumulates PSUM += I.T @ fw_k per slot (identity
    stationary weights: PE is a pure PSUM accumulator, ~40ns/slot).
    Epilogue in 4-chunk windows: batched max/reciprocal on the PSUM
    denominator column, per-chunk ACT scaled PSUM->SBUF copy, one batched
    residual add, one batched output DMA.
  - Softmax max-subtraction is skipped: e is O(10), exp() is safe in f32,
    and a = exp(e)/sum(exp(e)) is mathematically identical.
"""
import sys

sys.path.insert(0, "/opt/trn_rl_repo")

import numpy as np
import ml_dtypes
from bass_rust import AP
import concourse.bass as bass
import concourse.tile as tile
from concourse import bacc, mybir
from concourse.bass2jax import run_bass_via_pjrt

P = 128
N_NODES = 100000
N_EDGES = 1600000
D = 64
WCOL = D + 2                                  # [ft(64) | 1 | 1]
N_CORES = 8
NODES_PER_CORE = N_NODES // N_CORES           # 12500
CHUNKS = (NODES_PER_CORE + P - 1) // P        # 98
GRID = CHUNKS * P                             # 12544 (44 pad)
T1_TILES = CHUNKS
T1_GRID = T1_TILES * P
NEG_SLOPE = 0.2
WIN = 4                                       # epilogue window (chunks)
BF16 = ml_dtypes.bfloat16

_cache = {}


def _build_program1():
    """T-build: tout2[66, n] = [ft | el | er].T for the core's node slice.

    One matmul per 128-node tile with a FIXED stationary [W | wl | wr]
    (loaded once); 512-col PSUM banks drained by ACT in 4-tile batches.
    """
    nc = bacc.Bacc("TRN2", target_bir_lowering=False, debug=False,
                   num_devices=N_CORES)
    featT = nc.dram_tensor("featT", [D, T1_GRID], mybir.dt.float32,
                           kind="ExternalInput")
    wall = nc.dram_tensor("wall", [D, D + 2], mybir.dt.float32,
                          kind="ExternalInput")
    tout2 = nc.dram_tensor("tout2", [D + 2, T1_GRID], mybir.dt.float32,
                           kind="ExternalOutput")
    with tile.TileContext(nc) as tc:
        with (tc.tile_pool(name="sb", bufs=3) as sb,
              tc.tile_pool(name="ob", bufs=3) as ob,
              tc.tile_pool(name="ps", bufs=4, space="PSUM") as ps,
              tc.tile_pool(name="pers", bufs=1) as pers):
            w_t = pers.tile([D, D + 2], mybir.dt.float32)
            nc.sync.dma_start(w_t[:], wall[:, :])
            B = 4                               # tiles per PSUM bank
            for t0 in range(0, T1_TILES, B):
                nb = min(B, T1_TILES - t0)
                ftT = sb.tile([D, B * P], mybir.dt.float32, tag="ftT")
                nc.sync.dma_start(ftT[:, 0:nb * P],
                                  featT[:, t0 * P:(t0 + nb) * P])
                psum = ps.tile([D + 2, 512], mybir.dt.float32, space="PSUM",
                               tag="t")
                nc.tensor.matmul(psum[:, 0:nb * P], lhsT=w_t[:],
                                 rhs=ftT[:, 0:nb * P],
                                 start=True, stop=True)
                row = ob.tile([D + 2, B * P], mybir.dt.float32, tag="row")
                nc.scalar.copy(row[:, 0:nb * P], psum[:, 0:nb * P])
                nc.sync.dma_start(tout2[:, t0 * P:(t0 + nb) * P],
                                  row[:, 0:nb * P])
    nc.finalize()
    return nc


def _build_program2(slot_counts):
    """Identity-PE aggregation pass. slot_counts[ch] = slots in chunk ch."""
    slot_counts = [int(s) for s in slot_counts]
    stot = sum(slot_counts)
    s2 = 2 * stot
    nc = bacc.Bacc("TRN2", target_bir_lowering=False, debug=False,
                   num_devices=N_CORES)
    rows = nc.dram_tensor("rows", [P, stot * WCOL], mybir.dt.bfloat16,
                          kind="ExternalInput")
    el2 = nc.dram_tensor("el2", [P, s2], mybir.dt.float16,
                         kind="ExternalInput")
    er2 = nc.dram_tensor("er2", [P, s2], mybir.dt.float16,
                         kind="ExternalInput")
    idn = nc.dram_tensor("idn", [P, P], mybir.dt.bfloat16,
                         kind="ExternalInput")
    fres = nc.dram_tensor("fres", [CHUNKS, P, D], mybir.dt.bfloat16,
                          kind="ExternalInput")
    out = nc.dram_tensor("out", [CHUNKS, P, D], mybir.dt.bfloat16,
                         kind="ExternalOutput")
    # chunk -> slot offset; x-tile boundaries aligned to chunk starts
    s_off = [0]
    for kk in slot_counts:
        s_off.append(s_off[-1] + kk)
    XT = 512                       # x-tile max width (slots)
    RG = WIN                       # rows-DMA group (chunks)
    rbounds = [0, 1, 2, 4, 8]
    while rbounds[-1] < CHUNKS:
        rbounds.append(min(rbounds[-1] + RG, CHUNKS))
    # x-tiles = unions of rows-groups (staircase targets, then XT)
    xtiles = []
    targets = [64, 192, 448]
    ti = 0
    i = 0
    while i < len(rbounds) - 1:
        tgt = targets[ti] if ti < len(targets) else XT
        ti += 1
        j = i + 1
        while (j < len(rbounds) - 1
               and s_off[rbounds[j]] - s_off[rbounds[i]] < tgt):
            j += 1
        xtiles.append((rbounds[i], rbounds[j]))
        i = j
    xt_max = max(s_off[b] - s_off[a] for a, b in xtiles)

    with tile.TileContext(nc) as tc:
        with (tc.tile_pool(name="rp", bufs=4) as rp,
              tc.tile_pool(name="fp", bufs=4) as fp,
              tc.tile_pool(name="xp", bufs=4) as xp,
              tc.tile_pool(name="ep", bufs=3) as ep,
              tc.tile_pool(name="wp", bufs=3) as wp,
              tc.tile_pool(name="ps", bufs=8, space="PSUM") as ps,
              tc.tile_pool(name="sp", bufs=3) as sp,
              tc.tile_pool(name="pers", bufs=1) as pers):
            i_t = pers.tile([P, P], mybir.dt.bfloat16)
            nc.sync.dma_start(i_t[:], idn[:, :])
            el_t = pers.tile([P, s2], mybir.dt.float16)
            er_t = pers.tile([P, s2], mybir.dt.float16)
            hd = min(256, s2)
            nc.sync.dma_start(el_t[:, 0:hd], el2[:, 0:hd])
            nc.sync.dma_start(er_t[:, 0:hd], er2[:, 0:hd])
            h = s2 // 2
            nc.sync.dma_start(el_t[:, hd:h], el2[:, hd:h])
            nc.scalar.dma_start(el_t[:, h:s2], el2[:, h:s2])
            nc.sync.dma_start(er_t[:, hd:h], er2[:, hd:h])
            nc.scalar.dma_start(er_t[:, h:s2], er2[:, h:s2])
            fres_t = pers.tile([P, CHUNKS * D], mybir.dt.bfloat16)
            fsrc_all = AP(fres[0].tensor, 0, [[D, P], [P * D, CHUNKS], [1, D]])
            nc.scalar.dma_start(fres_t[:], fsrc_all)

            cur_e = [None, -1]       # (tile, xtile idx)

            def emit_xtile(ti):
                """e = leaky_relu(el2 + er2) for the tile's slot range."""
                clo, chi = xtiles[ti]
                t0, t1 = 2 * s_off[clo], 2 * s_off[chi]
                tw = t1 - t0
                e_s = ep.tile([P, 2 * xt_max], mybir.dt.float16, tag="e")
                nc.vector.tensor_add(e_s[:, 0:tw], el_t[:, t0:t1],
                                     er_t[:, t0:t1])
                nc.vector.scalar_tensor_tensor(
                    out=e_s[:, 0:tw], in0=e_s[:, 0:tw], scalar=NEG_SLOPE,
                    in1=e_s[:, 0:tw],
                    op0=mybir.AluOpType.mult, op1=mybir.AluOpType.max)
                x2_s = xp.tile([P, 2 * xt_max], mybir.dt.bfloat16, tag="x2")
                nc.scalar.activation(x2_s[:, 0:tw], e_s[:, 0:tw],
                                     mybir.ActivationFunctionType.Exp)
                return x2_s

            def epilogue(w0, w_psums, rec_w):
                """Batched scale+residual for chunks w0..w0+nw-1."""
                nw = len(w_psums)
                t_w = sp.tile([P, WIN * D], mybir.dt.bfloat16, tag="t")
                for i, psum in enumerate(w_psums):
                    nc.scalar.activation(t_w[:, i * D:(i + 1) * D],
                                         psum[:, 0:D],
                                         mybir.ActivationFunctionType.Copy,
                                         scale=rec_w[:, i:i + 1])
                o_w = sp.tile([P, WIN * D], mybir.dt.bfloat16, tag="o")
                nc.vector.tensor_add(o_w[:, 0:nw * D], t_w[:, 0:nw * D],
                                     fres_t[:, w0 * D:(w0 + nw) * D])
                odst = AP(out[0].tensor, w0 * P * D,
                          [[D, P], [P * D, nw], [1, D]])
                nc.gpsimd.dma_start(odst, o_w[:, 0:nw * D])

            # lag-1 window pipeline: emit window w's epilogue after window
            # w+1's multiplies so no engine waits on the PE in-line
            ready = None
            pend_w0 = None
            pend_psums = []
            den_w = None
            pend_e = None
            next_xt = 0
            rg_max = max(s_off[b] - s_off[a]
                         for a, b in zip(rbounds, rbounds[1:]))
            rstarts = set(rbounds[:-1])
            rg_bound = dict(zip(rbounds, rbounds[1:]))
            rg_tile = None
            fw_g = None
            rg_lo = -1
            for ch in range(CHUNKS):
                kk = slot_counts[ch]
                s0 = s_off[ch]
                while next_xt < len(xtiles) and xtiles[next_xt][0] <= ch + 6:
                    nxt = [emit_xtile(next_xt), next_xt]
                    if next_xt == 0:
                        cur_e = nxt
                    else:
                        pend_e = nxt
                    next_xt += 1
                if cur_e[1] >= 0 and ch >= xtiles[cur_e[1]][1]:
                    cur_e = pend_e
                if ch in rstarts:
                    glo, ghi = ch, rg_bound[ch]
                    gk = s_off[ghi] - s_off[glo]
                    gw = gk * WCOL
                    rg_tile = rp.tile([P, rg_max * WCOL],
                                      mybir.dt.bfloat16, tag="rows")
                    nc.sync.dma_start(
                        rg_tile[:, 0:gw],
                        rows[:, s_off[glo] * WCOL:s_off[ghi] * WCOL])
                    rg_lo = glo
                    # one wide fw = rows * x_broadcast for the whole group
                    x2g = cur_e[0]
                    xlo = xtiles[cur_e[1]][0]
                    goff = 2 * (s_off[glo] - s_off[xlo])
                    fw_g = fp.tile([P, rg_max * WCOL], mybir.dt.bfloat16,
                                   tag="fw")
                    xbg = AP(x2g[:].tensor, goff,
                             [[2 * xt_max, P], [2, gk], [0, WCOL // 2],
                              [1, 2]])
                    nc.vector.tensor_mul(fw_g[:, 0:gw],
                                         rg_tile[:, 0:gw], xbg)
                if pend_w0 is None:
                    pend_w0 = ch
                    den_w = wp.tile([P, WIN], mybir.dt.float32, tag="den")
                foff = (s0 - s_off[rg_lo]) * WCOL
                psum = ps.tile([P, 512], mybir.dt.float32, space="PSUM",
                               tag="acc")
                for k in range(kk):
                    nc.tensor.matmul(
                        psum[:, 0:WCOL], lhsT=i_t[:],
                        rhs=fw_g[:, foff + k * WCOL:foff + (k + 1) * WCOL],
                        start=(k == 0), stop=(k == kk - 1))
                # denominator (2*sum_k x) sits in PSUM col D (ones column)
                nc.scalar.copy(den_w[:, ch - pend_w0:ch - pend_w0 + 1],
                               psum[:, D:D + 1])
                pend_psums.append(psum)
                if len(pend_psums) == WIN or ch == CHUNKS - 1:
                    # rec = 1 / max(den/2, eps): depends only on the exps
                    nw = len(pend_psums)
                    nc.vector.tensor_scalar_max(den_w[:, 0:nw],
                                                den_w[:, 0:nw], 1e-30)
                    rec_w = wp.tile([P, WIN], mybir.dt.float32, tag="rec")
                    nc.vector.reciprocal(rec_w[:, 0:nw], den_w[:, 0:nw])
                    if ready is not None:
                        epilogue(*ready)
                    ready = (pend_w0, pend_psums, rec_w)
                    pend_w0, pend_psums = None, []
            if ready is not None:
                epilogue(*ready)
    nc.finalize()
    return nc


def _preprocess(src, dst):
    """Edge layout: per-core degree-sorted chunk/slot grid, common profile.

    Returns (perm[core][GRID] node-ids with -1 pads, slot_counts[CHUNKS],
    slot_src[core] int64 [total_slots, P] with -1 for pad slots).
    """
    deg = np.bincount(dst, minlength=N_NODES)
    order = np.argsort(dst, kind="stable")
    src_by_dst = src[order]
    rptr = np.zeros(N_NODES + 1, np.int64)
    np.cumsum(deg, out=rptr[1:])

    perms = []
    percore_counts = np.zeros((N_CORES, CHUNKS), np.int64)
    for c in range(N_CORES):
        lo = c * NODES_PER_CORE
        nodes = np.arange(lo, lo + NODES_PER_CORE)
        p = nodes[np.argsort(deg[nodes], kind="stable")]
        grid = np.full(GRID, -1, np.int64)
        grid[GRID - NODES_PER_CORE:] = p          # pads first (low-deg end)
        perms.append(grid)
        g = grid.reshape(CHUNKS, P)
        for ch in range(CHUNKS):
            real = g[ch][g[ch] >= 0]
            percore_counts[c, ch] = deg[real].max() if len(real) else 0
    slot_counts = np.maximum(1, percore_counts.max(axis=0))

    slot_srcs = []
    total = int(slot_counts.sum())
    for c in range(N_CORES):
        g = perms[c].reshape(CHUNKS, P)
        ss = np.full((total, P), -1, np.int64)
        s0 = 0
        for ch in range(CHUNKS):
            kk = int(slot_counts[ch])
            for p in range(P):
                n = g[ch, p]
                if n >= 0 and deg[n] > 0:
                    e = src_by_dst[rptr[n]:rptr[n + 1]]
                    ss[s0:s0 + len(e), p] = e
            s0 += kk
        slot_srcs.append(ss)
    return perms, slot_counts, slot_srcs


def _prepare(feat, W, attn_l, attn_r, bias, src, dst):
    """Preprocess + device program 1, build program-2 input maps."""
    feat = np.asarray(feat, dtype=np.float32)
    W = np.asarray(W, dtype=np.float32)
    attn_l = np.asarray(attn_l, dtype=np.float32).reshape(-1)
    attn_r = np.asarray(attn_r, dtype=np.float32).reshape(-1)
    bias = np.asarray(bias, dtype=np.float32).reshape(-1)
    src = np.asarray(src).astype(np.int64)
    dst = np.asarray(dst).astype(np.int64)

    perms, slot_counts, slot_srcs = _preprocess(src, dst)

    # ---- program 1: T = [ft | el | er] on device (8-way sharded) ----
    if "p1" not in _cache:
        _cache["p1"] = _build_program1()
    nc1 = _cache["p1"]

    featT_pad = np.zeros((D, N_CORES * T1_GRID), np.float32)
    featT_pad[:, :N_NODES] = feat.T
    wl = W @ attn_l
    wr = W @ attn_r
    wall = np.concatenate([W, wl[:, None], wr[:, None]],
                          axis=1).astype(np.float32)
    in_maps1 = []
    for c in range(N_CORES):
        in_maps1.append({
            "featT": np.ascontiguousarray(
                featT_pad[:, c * T1_GRID:(c + 1) * T1_GRID]),
            "wall": np.ascontiguousarray(wall),
        })
    res1 = run_bass_via_pjrt(nc1, in_maps1, N_CORES)
    T_full = np.concatenate([r["tout2"].T for r in res1], axis=0)[:N_NODES]
    # T_full: [N_NODES, 66] = [ft(64) | el | er]

    # ---- host: index-gather tables into per-core streams ----
    ftq = np.zeros((N_NODES + 1, WCOL), np.float32)
    ftq[:N_NODES, 0:D] = T_full[:, 0:D]
    ftq[:N_NODES, D:D + 2] = 1.0
    ftq = ftq.astype(BF16)
    el_tab = np.full(N_NODES + 1, -6e4, np.float32)
    el_tab[:N_NODES] = T_full[:, D]
    er_tab = np.zeros(N_NODES + 1, np.float32)
    er_tab[:N_NODES] = T_full[:, D + 1]
    fres_tab = np.zeros((N_NODES + 1, D), np.float32)
    fres_tab[:N_NODES] = feat + bias
    fres_tab = fres_tab.astype(BF16)
    idn = np.eye(P, dtype=np.float32).astype(BF16)

    stot = int(slot_counts.sum())
    in_maps2 = []
    for c in range(N_CORES):
        ss = slot_srcs[c]                          # [stot, P], -1 pads
        ssx = np.where(ss < 0, N_NODES, ss)
        rows = np.ascontiguousarray(
            ftq[ssx].transpose(1, 0, 2)).reshape(P, stot * WCOL)
        el_g = el_tab[ssx].T                       # [P, stot]
        el2 = np.repeat(el_g, 2, axis=1).astype(np.float16)
        gw = np.where(perms[c] < 0, N_NODES, perms[c])
        er_row = er_tab[gw].reshape(CHUNKS, P)     # [CHUNKS, P]
        er_g = np.repeat(er_row.T, np.asarray(slot_counts, np.int64),
                         axis=1)                   # [P, stot]
        er2 = np.repeat(er_g, 2, axis=1).astype(np.float16)
        in_maps2.append({
            "rows": rows,
            "el2": np.ascontiguousarray(el2),
            "er2": np.ascontiguousarray(er2),
            "idn": np.ascontiguousarray(idn),
            "fres": np.ascontiguousarray(fres_tab[gw].reshape(CHUNKS, P, D)),
        })
    return perms, slot_counts, in_maps2


def kernel(feat, W, attn_l, attn_r, bias, src, dst):
    perms, slot_counts, in_maps2 = _prepare(feat, W, attn_l, attn_r,
                                            bias, src, dst)
    key2 = ("p2", tuple(int(x) for x in slot_counts))
    if key2 not in _cache:
        _cache[key2] = _build_program2(slot_counts)
    res2 = run_bass_via_pjrt(_cache[key2], in_maps2, N_CORES)

    # ---- unshard ----
    rst = np.zeros((N_NODES, D), np.float32)
    for c in range(N_CORES):
        o = res2[c]["out"].reshape(GRID, D).astype(np.float32)
        g = perms[c]
        mask = g >= 0
        rst[g[mask]] = o[mask]
    return rst.reshape(N_NODES, 1, D)
